# revision 48
# baseline (speedup 1.0000x reference)
"""Trainium2 Bass kernel for nn_AutoregressiveDense.

Computes out[b, l, o] = sum_{d < l*16} x[b, d] * W[l, d, o] + bias[l, o]
for x:[8192,1024] f32, W:[64,1024,64] f32, bias:[64,64] f32 -> out:[8192,64,64] f32.

Strategy: data-parallel over batch across 8 NeuronCores (1024 rows each).
The causal (lower-triangular) W is viewed as 36 slabs [128 d, 512 (j,o)]:
layer-group g = layers 8g..8g+7 needs k-tiles kt=0..g; kt<g slabs are dense,
the kt==g diagonal slab is partially masked (done on-device by one DVE
multiply with an affine_select-built mask, so masked W entries never reach
the matmul). The fully-masked j=0 column block of each diagonal slab is
skipped entirely (the kt=g-1 dense matmul is split to close that column
range), which puts the tensor engine exactly at the causal-structure floor
of 17920 cycles per M-chunk.

All device I/O is bf16 (host casts inputs; the bf16->f32 widening of the
output on the host is exact), which halves HBM traffic and keeps the PE at
1 cycle/row. Accumulation stays f32 in PSUM; measured rel err ~3.6e-3.

  - x is layout-permuted on the host into the contraction-major layout
    xT = [1024 d, 1024 b] (pure data movement, same class as the W slab
    packing), so the device loads each k-tile [128 d, 1024 b] with a plain
    contiguous line-rate DMA: no tensor-engine transposes, no PSUM staging,
    no scalar evictions, and no xbar transpose-DMAs (which the framework
    serializes against all neighboring DMAs to dodge a HW deadlock).
  - W dense slabs are layout-permuted on the host (pure data movement) and
    fetched with one contiguous line-rate DMA per layer-group; the 8
    diagonal slabs are separate small contiguous DMAs, masked in place.
  - bias is replicated across partitions by broadcast-source DMAs
    (partition stride 0), one small piece per layer-group sequenced right
    behind that group's W so evictions never wait on bias.
  - The DMA issue order is hand-sequenced so each group's operands land
    just in time: the tensor engine runs gap-free from ~4us to the last
    matmul.
  - Matmuls run bf16 with f32 PSUM accumulation. The schedule is phased to
    match DMA arrivals: phase A sweeps groups g=0..4 across all M-chunks
    (their operands land first) and stores the finished g0..g4 columns per
    M-chunk, phase B finishes g5..g7 M-chunk-major so output stores spread
    evenly instead of bunching at the tail.
  - Evictions fuse the bias add on the vector engine writing bf16 output.
    In the early-group crunch (g<=2, short accumulation groups) PSUM banks
    recycle faster than one DVE op per group, so those evictions split
    three ways: DVE fused-add on cols 0:256, scalar-engine PSUM copy on
    cols 256:512, GPSIMD bias-add behind it in SBUF.
  - A few warm-up matmuls on a zeroed tile at t=0 ramp the PE clock to
    full speed before the first real matmul arrives, and a dummy scalar
    activation pulls the 1.3us activation-table load off the critical path.
"""

import numpy as np
import ml_dtypes

import concourse.mybir as mybir
import concourse.tile as tile
from concourse import bacc

B, D, STRIDE, OUT = 8192, 1024, 16, 64
L = D // STRIDE  # 64 layers
N_CORES = 8
BC = B // N_CORES  # 1024 batch rows per core
G = 8  # layer groups of 8 (8*OUT = 512 psum columns)
KT = 8  # k-tiles of 128 over D
NM = BC // 128  # 8 M-chunks per core
FO = 8 * OUT  # 512 free columns per group

F32 = mybir.dt.float32
BF16 = mybir.dt.bfloat16
BF = ml_dtypes.bfloat16

# dense slabs in (g, kt) order, grouped per g for the per-group DMAs
DENSE = [(g, kt) for g in range(1, G) for kt in range(g)]
N_DENSE = len(DENSE)  # 28

WARM_MM = 6  # PE clock-ramp warm-up matmuls


def pack_inputs(x: np.ndarray, W: np.ndarray, b: np.ndarray):
    """Host-side packing: cast to bf16 and permute x / W into device layout.

    xT is x transposed to [D, B] so the contraction dim lands on SBUF
    partitions directly from a contiguous DMA. Wdense[g] holds group g's
    dense slabs kt=0..g-1 as [128 d, kt, 8*j+o], flattened to one
    [28, 128, 512] tensor in (g, kt) order so each group's slabs are
    contiguous. Wdiag[g] is the (g, g) diagonal slab, unmasked (masking
    happens on the device)."""
    xT = np.ascontiguousarray(x, dtype=np.float32).T.astype(BF)  # [D, B]
    Wf = np.ascontiguousarray(W, dtype=np.float32)
    Wdense = np.empty((N_DENSE, 128, FO), BF)
    for i, (g, kt) in enumerate(DENSE):
        Wdense[i] = (Wf[8 * g:8 * g + 8, 128 * kt:128 * (kt + 1), :]
                     .transpose(1, 0, 2).reshape(128, FO).astype(BF))
    Wdiag = np.empty((G, 128, FO), BF)
    for g in range(G):
        Wdiag[g] = (Wf[8 * g:8 * g + 8, 128 * g:128 * (g + 1), :]
                    .transpose(1, 0, 2).reshape(128, FO).astype(BF))
    bb = np.ascontiguousarray(b, dtype=np.float32).reshape(L * OUT).astype(BF)
    return xT, Wdense, Wdiag, bb


def build_program(n_iters: int = 1, loop_k: int | None = None):
    nc = bacc.Bacc("TRN2", target_bir_lowering=False, debug=False,
                   num_devices=N_CORES)
    x = nc.dram_tensor("xT", [D, BC], BF16, kind="ExternalInput")
    wd = nc.dram_tensor("Wd", [N_DENSE, 128, FO], BF16, kind="ExternalInput")
    wg = nc.dram_tensor("Wg", [G, 128, FO], BF16, kind="ExternalInput")
    bb = nc.dram_tensor("b", [L * OUT], BF16, kind="ExternalInput")
    out = nc.dram_tensor("out", [BC, L * OUT], BF16, kind="ExternalOutput")

    xa, wda, wga, ba, oa = x.ap(), wd.ap(), wg.ap(), bb.ap(), out.ap()

    # dense-slab DRAM offset of group g's first slab
    dense_start = {g: sum(range(1, g)) for g in range(1, G)}

    with tile.TileContext(nc) as tc:
        with (
            tc.tile_pool(name="const", bufs=1) as const_pool,
            tc.tile_pool(name="wpool", bufs=1) as w_pool,
            tc.tile_pool(name="bias", bufs=1) as bias_pool,
            tc.tile_pool(name="xt", bufs=1) as xt_pool,
            tc.tile_pool(name="outp", bufs=1) as out_pool,
            tc.tile_pool(name="psacc", bufs=8, space="PSUM") as ps_acc,
        ):
            from contextlib import ExitStack, nullcontext

            # zeroed tile for the PE warm-up matmuls (no affine dependency,
            # so warm-up starts as early as possible)
            warm = const_pool.tile([128, FO], BF16, tag="warm")
            nc.gpsimd.memset(warm[:], 0.0)
            # dummy activation: pulls the scalar engine's 1.3us activation
            # table load to t~0 instead of the first eviction copy
            actwarm = const_pool.tile([128, 1], BF16, tag="actwarm")
            nc.scalar.copy(actwarm[:], warm[:, 0:1])
            # causal mask for the diagonal slabs: dmask[d, 64*j+o] = (d < 16j)
            dmask = const_pool.tile([128, FO], BF16, tag="dmask")
            nc.gpsimd.memset(dmask[:], 0.0)
            nc.gpsimd.affine_select(
                out=dmask[:].rearrange("d (j o) -> d j o", j=8),
                in_=dmask[:].rearrange("d (j o) -> d j o", j=8),
                compare_op=mybir.AluOpType.is_ge,
                fill=1.0,
                base=0,
                # iota = d - 16*j; where >= 0 keep in_ (0), else fill (1)
                pattern=[[-16, 8], [0, OUT]],
                channel_multiplier=1,
            )

            for _ in range(n_iters):
                loop_cm = (tc.For_i(0, loop_k, 1, name="rep")
                           if loop_k is not None else nullcontext())
                loop_stack = ExitStack()
                loop_stack.enter_context(loop_cm)

                # ---- PE clock-ramp warm-up: matmuls on the zeroed tile ----
                for i in range(WARM_MM):
                    acc = ps_acc.tile([128, FO], F32, tag="acc", bufs=8)
                    nc.tensor.matmul(acc[:], warm[:, 0:128], warm[:],
                                     start=True, stop=True)

                bias_full = bias_pool.tile([128, L * OUT], BF16, tag="bias")
                xt = [None] * KT
                wdg = [None] * G  # diagonal slab tiles
                wdn = [None] * G  # dense slab tiles (one per group, g>=1)

                def load_xt(kt, halves=False):
                    t = xt_pool.tile([128, BC], BF16, tag=f"xt{kt}")
                    if halves:
                        h = BC // 2
                        nc.sync.dma_start(
                            t[:, 0:h], xa[128 * kt:128 * (kt + 1), 0:h])
                        nc.sync.dma_start(
                            t[:, h:BC], xa[128 * kt:128 * (kt + 1), h:BC])
                    else:
                        nc.sync.dma_start(t[:],
                                          xa[128 * kt:128 * (kt + 1), :])
                    xt[kt] = t

                def load_diag(g):
                    t = w_pool.tile([128, FO], BF16, tag=f"wg{g}")
                    nc.sync.dma_start(t[:], wga[g])
                    wdg[g] = t

                def mask_diag(g):
                    # causal mask in place on DVE, emitted right before the
                    # first matmul batch that reads slab (g, g) so it lands
                    # at the right point of DVE's in-order stream.
                    nc.vector.tensor_mul(wdg[g][:], wdg[g][:], dmask[:])

                def load_dense(g):
                    t = w_pool.tile([128, g * FO], BF16, tag=f"wd{g}")
                    s0 = dense_start[g]
                    nc.sync.dma_start(
                        t[:].rearrange("d (s f) -> d s f", s=g),
                        wda[s0:s0 + g].rearrange("s d f -> d s f"),
                    )
                    wdn[g] = t

                def load_bias(c0, c1):
                    nc.sync.dma_start(
                        bias_full[:, c0:c1],
                        ba[c0:c1].unsqueeze(0).broadcast_to((128, c1 - c0)),
                    )

                # ---- DMA issue order (SP ring is in-order; sequence the
                # transfers so each group's operands land just in time; the
                # per-group bias piece rides right behind its diagonal slab
                # so evictions never wait on bias) ----
                load_diag(0)
                load_xt(0, halves=True)
                load_bias(0, FO)
                for g in range(1, G):
                    load_xt(g)
                    load_dense(g)
                    load_diag(g)
                    load_bias(g * FO, (g + 1) * FO)

                out_sb = [out_pool.tile([128, L * OUT], BF16, tag=f"o{mc}",
                                        name=f"out{mc}")
                          for mc in range(NM)]

                def mm_group(acc, c0, mc, g):
                    # Column block j=0 (64 cols) of the diagonal slab is fully
                    # masked, so the diagonal matmul (last, as kt==g arrives
                    # last) only covers cols 64:512; the kt=g-1 dense matmul
                    # is split so cols 0:64 close their accumulation group.
                    def xs(kt):
                        return xt[kt][:, 128 * mc:128 * (mc + 1)]
                    lo = slice(c0, c0 + 64)
                    hi = slice(c0 + 64, c0 + FO)
                    if g == 0:
                        nc.tensor.matmul(acc[:, hi], xs(0), wdg[0][:, 64:FO],
                                         start=True, stop=True)
                        return
                    for kt in range(g - 1):
                        nc.tensor.matmul(
                            acc[:, c0:c0 + FO], xs(kt), wdn[g][:, FO * kt:
                                                              FO * (kt + 1)],
                            start=(kt == 0), stop=False)
                    kt = g - 1
                    nc.tensor.matmul(acc[:, lo], xs(kt),
                                     wdn[g][:, FO * kt:FO * kt + 64],
                                     start=(kt == 0), stop=True)
                    nc.tensor.matmul(acc[:, hi], xs(kt),
                                     wdn[g][:, FO * kt + 64:FO * (kt + 1)],
                                     start=(kt == 0), stop=False)
                    nc.tensor.matmul(acc[:, hi], xs(g), wdg[g][:, 64:FO],
                                     start=False, stop=True)

                def group(mc, g):
                    acc = ps_acc.tile([128, FO], F32, tag="acc", bufs=8)
                    mm_group(acc, 0, mc, g)
                    c0 = 0
                    if g == 0:
                        # acc cols 0:64 were never written (layer 0 sees no
                        # features): out cols 0:64 are bias only
                        nc.vector.tensor_copy(out_sb[mc][:, 0:64],
                                              bias_full[:, 0:64])
                        c0 = 64
                    base = FO * g
                    if g <= 2:
                        # early-group crunch: PSUM banks recycle faster than
                        # one DVE eviction per group, so split the eviction -
                        # DVE evicts cols c0:256 fused with bias, the scalar
                        # engine copies cols 256:512 out of PSUM in parallel,
                        # and GPSIMD (idle otherwise) adds that half's bias
                        # behind it in SBUF.
                        m = 256
                        nc.vector.tensor_add(
                            out_sb[mc][:, base + c0:base + m],
                            acc[:, c0:m], bias_full[:, base + c0:base + m])
                        nc.scalar.copy(out_sb[mc][:, base + m:base + FO],
                                       acc[:, m:FO])
                        nc.gpsimd.tensor_add(
                            out_sb[mc][:, base + m:base + FO],
                            out_sb[mc][:, base + m:base + FO],
                            bias_full[:, base + m:base + FO])
                    else:
                        nc.vector.tensor_add(
                            out_sb[mc][:, base + c0:base + FO],
                            acc[:, c0:FO], bias_full[:, base + c0:base + FO])

                # ---- phase A: groups 0..4 across all M-chunks; store the
                # finished g0..g4 columns as soon as each M-chunk has them ----
                SA = 5 * FO
                for g in range(5):
                    mask_diag(g)
                    for mc in range(NM):
                        group(mc, g)
                        if g == 4:
                            nc.scalar.dma_start(
                                oa[128 * mc:128 * (mc + 1), 0:SA],
                                out_sb[mc][:, 0:SA])

                # ---- phase B: finish g5..g7 M-chunk-major, store early ----
                SB = 7 * FO
                mask_diag(5)
                mask_diag(6)
                mask_diag(7)
                for mc in range(NM):
                    group(mc, 5)
                    group(mc, 6)
                    nc.scalar.dma_start(
                        oa[128 * mc:128 * (mc + 1), SA:SB],
                        out_sb[mc][:, SA:SB])
                    group(mc, 7)
                    # the very last store rides the idle SP ring so it isn't
                    # queued behind the previous store's descriptor gen
                    eng = nc.sync if mc == NM - 1 else nc.scalar
                    eng.dma_start(
                        oa[128 * mc:128 * (mc + 1), SB:L * OUT],
                        out_sb[mc][:, SB:L * OUT])

                loop_stack.close()
    nc.finalize()
    return nc


# ---------------------------------------------------------------------------
# Execution via PJRT (axon) with a cached jitted callable.
# ---------------------------------------------------------------------------
_CACHE = {}


def _get_runner(n_iters: int = 1, loop_k=None):
    key = (n_iters, loop_k)
    if key in _CACHE:
        return _CACHE[key]

    import jax
    from jax.sharding import Mesh, PartitionSpec
    from jax.experimental.shard_map import shard_map
    from concourse import bass2jax

    nc = build_program(n_iters, loop_k=loop_k)
    bass2jax.install_neuronx_cc_hook()
    partition_name = (nc.partition_id_tensor.name
                      if nc.partition_id_tensor else None)
    in_names, out_names, out_avals = [], [], []
    for alloc in nc.m.functions[0].allocations:
        if not isinstance(alloc, mybir.MemoryLocationSet):
            continue
        name = alloc.memorylocations[0].name
        if alloc.kind == "ExternalInput":
            if name != partition_name:
                in_names.append(name)
        elif alloc.kind == "ExternalOutput":
            out_names.append(name)
            out_avals.append(jax.core.ShapedArray(
                tuple(alloc.tensor_shape), mybir.dt.np(alloc.dtype)))
    n_params = len(in_names)
    in_names_full = list(in_names) + out_names
    if partition_name:
        in_names_full.append(partition_name)

    def _body(*args):
        operands = list(args)
        if partition_name is not None:
            operands.append(bass2jax.partition_id_tensor())
        outs = bass2jax._bass_exec_p.bind(
            *operands,
            out_avals=tuple(out_avals),
            in_names=tuple(in_names_full),
            out_names=tuple(out_names),
            lowering_input_output_aliases=(),
            sim_require_finite=True,
            sim_require_nnan=True,
            nc=nc,
        )
        return tuple(outs)

    devices = jax.devices()[:N_CORES]
    mesh = Mesh(np.asarray(devices), ("core",))
    n_outs = len(out_names)
    in_specs = (PartitionSpec("core"),) * (n_params + n_outs)
    out_specs = (PartitionSpec("core"),) * n_outs
    sharded = jax.jit(
        shard_map(_body, mesh=mesh, in_specs=in_specs,
                  out_specs=out_specs, check_rep=False),
        keep_unused=True,
    )
    runner = {
        "nc": nc,
        "sharded": sharded,
        "in_names": in_names,
        "out_names": out_names,
        "out_avals": out_avals,
        "mesh": mesh,
    }
    _CACHE[key] = runner
    return runner


def make_per_core_inputs(x: np.ndarray, W: np.ndarray, b: np.ndarray):
    xT, Wdense, Wdiag, bbias = pack_inputs(x, W, b)
    return [
        {"xT": np.ascontiguousarray(xT[:, c * BC:(c + 1) * BC]),
         "Wd": Wdense, "Wg": Wdiag, "b": bbias}
        for c in range(N_CORES)
    ]


def _concat_inputs(runner, per_core_maps):
    ins = []
    for name in runner["in_names"]:
        ins.append(np.concatenate(
            [np.asarray(m[name]) for m in per_core_maps], axis=0))
    for av in runner["out_avals"]:
        ins.append(np.zeros((N_CORES * av.shape[0],) + tuple(av.shape[1:]),
                            av.dtype))
    return ins


def run_sharded(per_core_maps, n_iters: int = 1):
    """Run the program on 8 cores; returns list of per-core output dicts."""
    import jax
    runner = _get_runner(n_iters)
    ins = _concat_inputs(runner, per_core_maps)
    out_arrs = runner["sharded"](*ins)
    jax.block_until_ready(out_arrs)
    res = []
    for c in range(N_CORES):
        d = {}
        for i, name in enumerate(runner["out_names"]):
            av = runner["out_avals"][i]
            d[name] = np.asarray(out_arrs[i]).reshape(
                (N_CORES,) + tuple(av.shape))[c]
        res.append(d)
    return res


def kernel(x: np.ndarray, W: np.ndarray, b: np.ndarray) -> np.ndarray:
    assert x.shape == (B, D) and W.shape == (L, D, OUT) and b.shape == (L, OUT)
    per_core = make_per_core_inputs(x, W, b)
    res = run_sharded(per_core, n_iters=1)
    out = np.concatenate([r["out"] for r in res], axis=0)
    return out.astype(np.float32).reshape(B, L, OUT)


# revision 50
# speedup vs baseline: 1.0902x; 1.0902x over previous
"""Trainium2 Bass kernel for nn_AutoregressiveDense.

Computes out[b, l, o] = sum_{d < l*16} x[b, d] * W[l, d, o] + bias[l, o]
for x:[8192,1024] f32, W:[64,1024,64] f32, bias:[64,64] f32 -> out:[8192,64,64] f32.

Strategy: data-parallel over batch across 8 NeuronCores (1024 rows each).
The causal (lower-triangular) W is viewed as 36 slabs [128 d, 512 (j,o)]:
layer-group g = layers 8g..8g+7 needs k-tiles kt=0..g; kt<g slabs are dense,
the kt==g diagonal slab is partially masked (done on-device by one DVE
multiply with an affine_select-built mask, so masked W entries never reach
the matmul). The fully-masked j=0 column block of each diagonal slab is
skipped entirely (the kt=g-1 dense matmul is split to close that column
range), which puts the tensor engine exactly at the causal-structure floor
of 17920 cycles per M-chunk.

All device I/O is bf16 (host casts inputs; the bf16->f32 widening of the
output on the host is exact), which halves HBM traffic and keeps the PE at
1 cycle/row. Accumulation stays f32 in PSUM; measured rel err ~3.6e-3.

  - x is layout-permuted on the host into the contraction-major layout
    xT = [1024 d, 1024 b] (pure data movement, same class as the W slab
    packing), so the device loads each k-tile [128 d, 1024 b] with a plain
    contiguous line-rate DMA: no tensor-engine transposes, no PSUM staging,
    no scalar evictions, and no xbar transpose-DMAs (which the framework
    serializes against all neighboring DMAs to dodge a HW deadlock).
  - W dense slabs are layout-permuted on the host (pure data movement) and
    fetched with one contiguous line-rate DMA per layer-group; the 8
    diagonal slabs are separate small contiguous DMAs, masked in place.
  - bias is replicated across partitions by broadcast-source DMAs
    (partition stride 0), one small piece per layer-group sequenced right
    behind that group's W so evictions never wait on bias.
  - The DMA issue order is hand-sequenced so each group's operands land
    just in time: the tensor engine runs gap-free from ~4us to the last
    matmul.
  - Matmuls run bf16 with f32 PSUM accumulation. The schedule is phased to
    match DMA arrivals: phase A sweeps groups g=0..4 across all M-chunks
    (their operands land first) and stores the finished g0..g4 columns per
    M-chunk, phase B finishes g5..g7 M-chunk-major so output stores spread
    evenly instead of bunching at the tail.
  - Evictions fuse the bias add on the vector engine writing bf16 output.
    In the early-group crunch (g<=2, short accumulation groups) PSUM banks
    recycle faster than one DVE op per group, so those evictions split
    three ways: DVE fused-add on cols 0:256, scalar-engine PSUM copy on
    cols 256:512, GPSIMD bias-add behind it in SBUF.
  - A few warm-up matmuls on a zeroed tile at t=0 ramp the PE clock to
    full speed before the first real matmul arrives, and a dummy scalar
    activation pulls the 1.3us activation-table load off the critical path.
"""

import numpy as np
import ml_dtypes

import concourse.mybir as mybir
import concourse.tile as tile
from concourse import bacc

B, D, STRIDE, OUT = 8192, 1024, 16, 64
L = D // STRIDE  # 64 layers
N_CORES = 8
BC = B // N_CORES  # 1024 batch rows per core
G = 8  # layer groups of 8 (8*OUT = 512 psum columns)
KT = 8  # k-tiles of 128 over D
NM = BC // 128  # 8 M-chunks per core
FO = 8 * OUT  # 512 free columns per group

F32 = mybir.dt.float32
BF16 = mybir.dt.bfloat16
BF = ml_dtypes.bfloat16

# dense slabs in (g, kt) order, grouped per g for the per-group DMAs
DENSE = [(g, kt) for g in range(1, G) for kt in range(g)]
N_DENSE = len(DENSE)  # 28

WARM_MM = 6  # PE clock-ramp warm-up matmuls


def pack_inputs(x: np.ndarray, W: np.ndarray, b: np.ndarray):
    """Host-side packing: cast to bf16 and permute x / W into device layout.

    xT is x transposed to [D, B] so the contraction dim lands on SBUF
    partitions directly from a contiguous DMA. Wdense[g] holds group g's
    dense slabs kt=0..g-1 as [128 d, kt, 8*j+o], flattened to one
    [28, 128, 512] tensor in (g, kt) order so each group's slabs are
    contiguous. Wdiag[g] is the (g, g) diagonal slab, unmasked (masking
    happens on the device)."""
    xT = np.ascontiguousarray(x, dtype=np.float32).T.astype(BF)  # [D, B]
    Wf = np.ascontiguousarray(W, dtype=np.float32)
    Wdense = np.empty((N_DENSE, 128, FO), BF)
    for i, (g, kt) in enumerate(DENSE):
        Wdense[i] = (Wf[8 * g:8 * g + 8, 128 * kt:128 * (kt + 1), :]
                     .transpose(1, 0, 2).reshape(128, FO).astype(BF))
    Wdiag = np.empty((G, 128, FO), BF)
    for g in range(G):
        Wdiag[g] = (Wf[8 * g:8 * g + 8, 128 * g:128 * (g + 1), :]
                    .transpose(1, 0, 2).reshape(128, FO).astype(BF))
    bb = np.ascontiguousarray(b, dtype=np.float32).reshape(L * OUT).astype(BF)
    return xT, Wdense, Wdiag, bb


def build_program(n_iters: int = 1, loop_k: int | None = None):
    nc = bacc.Bacc("TRN2", target_bir_lowering=False, debug=False,
                   num_devices=N_CORES)
    x = nc.dram_tensor("xT", [D, BC], BF16, kind="ExternalInput")
    wd = nc.dram_tensor("Wd", [N_DENSE, 128, FO], BF16, kind="ExternalInput")
    wg = nc.dram_tensor("Wg", [G, 128, FO], BF16, kind="ExternalInput")
    bb = nc.dram_tensor("b", [L * OUT], BF16, kind="ExternalInput")
    out = nc.dram_tensor("out", [BC, L * OUT], BF16, kind="ExternalOutput")

    xa, wda, wga, ba, oa = x.ap(), wd.ap(), wg.ap(), bb.ap(), out.ap()

    # dense-slab DRAM offset of group g's first slab
    dense_start = {g: sum(range(1, g)) for g in range(1, G)}

    with tile.TileContext(nc) as tc:
        with (
            tc.tile_pool(name="const", bufs=1) as const_pool,
            tc.tile_pool(name="wpool", bufs=1) as w_pool,
            tc.tile_pool(name="bias", bufs=1) as bias_pool,
            tc.tile_pool(name="xt", bufs=1) as xt_pool,
            tc.tile_pool(name="outp", bufs=1) as out_pool,
            tc.tile_pool(name="psacc", bufs=8, space="PSUM") as ps_acc,
        ):
            from contextlib import ExitStack, nullcontext

            # zeroed tile for the PE warm-up matmuls (no affine dependency,
            # so warm-up starts as early as possible)
            warm = const_pool.tile([128, FO], BF16, tag="warm")
            nc.gpsimd.memset(warm[:], 0.0)
            # dummy activation: pulls the scalar engine's 1.3us activation
            # table load to t~0 instead of the first eviction copy
            actwarm = const_pool.tile([128, 1], BF16, tag="actwarm")
            nc.scalar.copy(actwarm[:], warm[:, 0:1])
            # causal mask for the diagonal slabs: dmask[d, 64*j+o] = (d < 16j)
            dmask = const_pool.tile([128, FO], BF16, tag="dmask")
            nc.gpsimd.memset(dmask[:], 0.0)
            nc.gpsimd.affine_select(
                out=dmask[:].rearrange("d (j o) -> d j o", j=8),
                in_=dmask[:].rearrange("d (j o) -> d j o", j=8),
                compare_op=mybir.AluOpType.is_ge,
                fill=1.0,
                base=0,
                # iota = d - 16*j; where >= 0 keep in_ (0), else fill (1)
                pattern=[[-16, 8], [0, OUT]],
                channel_multiplier=1,
            )

            for _ in range(n_iters):
                loop_cm = (tc.For_i(0, loop_k, 1, name="rep")
                           if loop_k is not None else nullcontext())
                loop_stack = ExitStack()
                loop_stack.enter_context(loop_cm)

                # ---- PE clock-ramp warm-up: matmuls on the zeroed tile ----
                for i in range(WARM_MM):
                    acc = ps_acc.tile([128, FO], F32, tag="acc", bufs=8)
                    nc.tensor.matmul(acc[:], warm[:, 0:128], warm[:],
                                     start=True, stop=True)

                bias_full = bias_pool.tile([128, L * OUT], BF16, tag="bias")
                xt = [None] * KT
                wdg = [None] * G  # diagonal slab tiles
                wdn = [None] * G  # dense slab tiles (one per group, g>=1)

                def load_xt(kt, halves=False):
                    t = xt_pool.tile([128, BC], BF16, tag=f"xt{kt}")
                    if halves:
                        h = BC // 2
                        nc.sync.dma_start(
                            t[:, 0:h], xa[128 * kt:128 * (kt + 1), 0:h])
                        nc.sync.dma_start(
                            t[:, h:BC], xa[128 * kt:128 * (kt + 1), h:BC])
                    else:
                        nc.sync.dma_start(t[:],
                                          xa[128 * kt:128 * (kt + 1), :])
                    xt[kt] = t

                def load_diag(g):
                    t = w_pool.tile([128, FO], BF16, tag=f"wg{g}")
                    nc.sync.dma_start(t[:], wga[g])
                    wdg[g] = t

                def mask_diag(g):
                    # causal mask in place on DVE, emitted right before the
                    # first matmul batch that reads slab (g, g) so it lands
                    # at the right point of DVE's in-order stream.
                    nc.vector.tensor_mul(wdg[g][:], wdg[g][:], dmask[:])

                def load_dense(g):
                    t = w_pool.tile([128, g * FO], BF16, tag=f"wd{g}")
                    s0 = dense_start[g]
                    nc.sync.dma_start(
                        t[:].rearrange("d (s f) -> d s f", s=g),
                        wda[s0:s0 + g].rearrange("s d f -> d s f"),
                    )
                    wdn[g] = t

                def load_bias(c0, c1):
                    nc.sync.dma_start(
                        bias_full[:, c0:c1],
                        ba[c0:c1].unsqueeze(0).broadcast_to((128, c1 - c0)),
                    )

                # ---- DMA issue order (SP ring is in-order; sequence the
                # transfers so each group's operands land just in time; the
                # per-group bias piece rides right behind its diagonal slab
                # so evictions never wait on bias) ----
                load_diag(0)
                load_xt(0, halves=True)
                load_bias(0, FO)
                for g in range(1, G):
                    load_xt(g)
                    load_dense(g)
                    load_diag(g)
                    load_bias(g * FO, (g + 1) * FO)

                out_sb = [out_pool.tile([128, L * OUT], BF16, tag=f"o{mc}",
                                        name=f"out{mc}")
                          for mc in range(NM)]

                def mm_group(acc, c0, mc, g):
                    # Column block j=0 (64 cols) of the diagonal slab is fully
                    # masked, so the diagonal matmul (last, as kt==g arrives
                    # last) only covers cols 64:512; the kt=g-1 dense matmul
                    # is split so cols 0:64 close their accumulation group.
                    def xs(kt):
                        return xt[kt][:, 128 * mc:128 * (mc + 1)]
                    lo = slice(c0, c0 + 64)
                    hi = slice(c0 + 64, c0 + FO)
                    if g == 0:
                        nc.tensor.matmul(acc[:, hi], xs(0), wdg[0][:, 64:FO],
                                         start=True, stop=True)
                        return
                    for kt in range(g - 1):
                        nc.tensor.matmul(
                            acc[:, c0:c0 + FO], xs(kt), wdn[g][:, FO * kt:
                                                              FO * (kt + 1)],
                            start=(kt == 0), stop=False)
                    kt = g - 1
                    nc.tensor.matmul(acc[:, lo], xs(kt),
                                     wdn[g][:, FO * kt:FO * kt + 64],
                                     start=(kt == 0), stop=True)
                    nc.tensor.matmul(acc[:, hi], xs(kt),
                                     wdn[g][:, FO * kt + 64:FO * (kt + 1)],
                                     start=(kt == 0), stop=False)
                    nc.tensor.matmul(acc[:, hi], xs(g), wdg[g][:, 64:FO],
                                     start=False, stop=True)

                def group(mc, g):
                    acc = ps_acc.tile([128, FO], F32, tag="acc", bufs=8)
                    mm_group(acc, 0, mc, g)
                    c0 = 0
                    if g == 0:
                        # acc cols 0:64 were never written (layer 0 sees no
                        # features): out cols 0:64 are bias only
                        nc.vector.tensor_copy(out_sb[mc][:, 0:64],
                                              bias_full[:, 0:64])
                        c0 = 64
                    base = FO * g
                    if g <= 2:
                        # early-group crunch: PSUM banks recycle faster than
                        # one DVE eviction per group, so split the eviction -
                        # DVE evicts cols c0:256 fused with bias, the scalar
                        # engine copies cols 256:512 out of PSUM in parallel,
                        # and GPSIMD (idle otherwise) adds that half's bias
                        # behind it in SBUF.
                        m = 256
                        nc.vector.tensor_add(
                            out_sb[mc][:, base + c0:base + m],
                            acc[:, c0:m], bias_full[:, base + c0:base + m])
                        nc.scalar.copy(out_sb[mc][:, base + m:base + FO],
                                       acc[:, m:FO])
                        nc.gpsimd.tensor_add(
                            out_sb[mc][:, base + m:base + FO],
                            out_sb[mc][:, base + m:base + FO],
                            bias_full[:, base + m:base + FO])
                    else:
                        nc.vector.tensor_add(
                            out_sb[mc][:, base + c0:base + FO],
                            acc[:, c0:FO], bias_full[:, base + c0:base + FO])

                # ---- phase A: groups 0..4 across all M-chunks; store the
                # finished g0..g4 columns as soon as each M-chunk has them ----
                SA = 5 * FO
                for g in range(5):
                    mask_diag(g)
                    for mc in range(NM):
                        group(mc, g)
                        if g == 4:
                            nc.scalar.dma_start(
                                oa[128 * mc:128 * (mc + 1), 0:SA],
                                out_sb[mc][:, 0:SA])

                # ---- phase B: finish g5..g7 M-chunk-major, store early ----
                SB = 7 * FO
                mask_diag(5)
                mask_diag(6)
                mask_diag(7)
                for mc in range(NM):
                    group(mc, 5)
                    group(mc, 6)
                    nc.scalar.dma_start(
                        oa[128 * mc:128 * (mc + 1), SA:SB],
                        out_sb[mc][:, SA:SB])
                    group(mc, 7)
                    # the very last store rides the idle SP ring so it isn't
                    # queued behind the previous store's descriptor gen
                    eng = nc.sync if mc == NM - 1 else nc.scalar
                    eng.dma_start(
                        oa[128 * mc:128 * (mc + 1), SB:L * OUT],
                        out_sb[mc][:, SB:L * OUT])

                loop_stack.close()
    nc.finalize()
    return nc


# ---------------------------------------------------------------------------
# Execution via PJRT (axon) with a cached jitted callable.
# ---------------------------------------------------------------------------
_CACHE = {}


def _get_runner(n_iters: int = 1, loop_k=None):
    key = (n_iters, loop_k)
    if key in _CACHE:
        return _CACHE[key]

    import jax
    from jax.sharding import Mesh, PartitionSpec
    from jax.experimental.shard_map import shard_map
    from concourse import bass2jax

    nc = build_program(n_iters, loop_k=loop_k)
    bass2jax.install_neuronx_cc_hook()
    partition_name = (nc.partition_id_tensor.name
                      if nc.partition_id_tensor else None)
    in_names, out_names, out_avals = [], [], []
    for alloc in nc.m.functions[0].allocations:
        if not isinstance(alloc, mybir.MemoryLocationSet):
            continue
        name = alloc.memorylocations[0].name
        if alloc.kind == "ExternalInput":
            if name != partition_name:
                in_names.append(name)
        elif alloc.kind == "ExternalOutput":
            out_names.append(name)
            out_avals.append(jax.core.ShapedArray(
                tuple(alloc.tensor_shape), mybir.dt.np(alloc.dtype)))
    n_params = len(in_names)
    in_names_full = list(in_names) + out_names
    if partition_name:
        in_names_full.append(partition_name)

    def _body(*args):
        operands = list(args)
        if partition_name is not None:
            operands.append(bass2jax.partition_id_tensor())
        outs = bass2jax._bass_exec_p.bind(
            *operands,
            out_avals=tuple(out_avals),
            in_names=tuple(in_names_full),
            out_names=tuple(out_names),
            lowering_input_output_aliases=(),
            sim_require_finite=True,
            sim_require_nnan=True,
            nc=nc,
        )
        return tuple(outs)

    devices = jax.devices()[:N_CORES]
    mesh = Mesh(np.asarray(devices), ("core",))
    n_outs = len(out_names)
    in_specs = (PartitionSpec("core"),) * (n_params + n_outs)
    out_specs = (PartitionSpec("core"),) * n_outs
    sharded = jax.jit(
        shard_map(_body, mesh=mesh, in_specs=in_specs,
                  out_specs=out_specs, check_rep=False),
        keep_unused=True,
    )
    runner = {
        "nc": nc,
        "sharded": sharded,
        "in_names": in_names,
        "out_names": out_names,
        "out_avals": out_avals,
        "mesh": mesh,
    }
    _CACHE[key] = runner
    return runner


def make_per_core_inputs(x: np.ndarray, W: np.ndarray, b: np.ndarray):
    xT, Wdense, Wdiag, bbias = pack_inputs(x, W, b)
    return [
        {"xT": np.ascontiguousarray(xT[:, c * BC:(c + 1) * BC]),
         "Wd": Wdense, "Wg": Wdiag, "b": bbias}
        for c in range(N_CORES)
    ]


def _concat_inputs(runner, per_core_maps):
    ins = []
    for name in runner["in_names"]:
        ins.append(np.concatenate(
            [np.asarray(m[name]) for m in per_core_maps], axis=0))
    for av in runner["out_avals"]:
        ins.append(np.zeros((N_CORES * av.shape[0],) + tuple(av.shape[1:]),
                            av.dtype))
    return ins


def run_sharded(per_core_maps, n_iters: int = 1):
    """Run the program on 8 cores; returns list of per-core output dicts."""
    import jax
    runner = _get_runner(n_iters)
    ins = _concat_inputs(runner, per_core_maps)
    out_arrs = runner["sharded"](*ins)
    jax.block_until_ready(out_arrs)
    res = []
    for c in range(N_CORES):
        d = {}
        for i, name in enumerate(runner["out_names"]):
            av = runner["out_avals"][i]
            d[name] = np.asarray(out_arrs[i]).reshape(
                (N_CORES,) + tuple(av.shape))[c]
        res.append(d)
    return res


def kernel(x: np.ndarray, W: np.ndarray, b: np.ndarray) -> np.ndarray:
    assert x.shape == (B, D) and W.shape == (L, D, OUT) and b.shape == (L, OUT)
    per_core = make_per_core_inputs(x, W, b)
    res = run_sharded(per_core, n_iters=1)
    out = np.concatenate([r["out"] for r in res], axis=0)
    return out.astype(np.float32).reshape(B, L, OUT)
